# revision 14
# baseline (speedup 1.0000x reference)
"""Trainium2 Bass kernel for BasicLSTM (nn_BasicLSTM_16320875724833).

Problem: inputs [256, 1024, 128] f32; LSTM(H=256) over T=1024 steps, then
linear [256->2] + softmax on the final hidden state. Output [256, 2] f32.

Strategy (8 cores, data-parallel over batch, 32 rows/core):
  - All state kept "transposed" (feature-major): hT/cT are [128p, 2, 32].
  - Gates for 4 consecutive steps accumulate in one PSUM tile
    [128p(gate-within-chunk), 8(gate-chunk), 4(step), 32(batch)]:
      1. bias matmul (K=4 indicator trick) fills bank with b_ih+b_hh, start=True
      2. input-projection matmuls (W_ih^T @ xT) accumulate, per gate-chunk
      3. per step, 16 recurrence matmuls (W_hh^T chunks @ hT) accumulate
  - x is cast to fp16 on host, DMA'd per-batch-row and transposed to
    d-major via the DMA xbar transpose engine.
  - Gate chunk order (host-side row permutation of the PyTorch [i,f,g,o]
    layout): [g0,g1, i0,i1, f0,f1, o0,o1] so tanh(g) is chunks 0:2 and
    sigmoid(i,f,o) is one contiguous ACT op over chunks 2:8.
  - Head: softmax over 2 classes == [sigmoid(d), sigmoid(-d)] with
    d = h @ (W_lin[0]-W_lin[1]) + (b_lin[0]-b_lin[1]).
"""

import numpy as np

# ---- problem constants (hardcoded; kernel.py must be self-contained) ----
B, T, D, H = 256, 1024, 128, 256
NCORES = 8
BLOC = B // NCORES          # 32 batch rows per core
GC = 8                      # gate chunks of 128 (4H = 1024)
KC = 2                      # hidden chunks of 128 (H = 256)
G4 = 4                      # timesteps per PSUM group
TC = 128                    # timesteps per transposed-x chunk

RECUR_F16 = True            # fp16 recurrence weights/state (False -> fp32)

_cache = {}


def _build_program(seq_len=T):
    import concourse.bass as bass
    import concourse.mybir as mybir
    from concourse import bacc
    from concourse.tile import TileContext
    from contextlib import ExitStack

    f16 = mybir.dt.float16
    f32 = mybir.dt.float32
    rdt = f16 if RECUR_F16 else f32
    AF = mybir.ActivationFunctionType

    nc = bacc.Bacc(None, target_bir_lowering=False)

    x = nc.dram_tensor("x", [BLOC, seq_len, D], f16, kind="ExternalInput")
    wih = nc.dram_tensor("wih", [D, 4 * H], f16, kind="ExternalInput")
    whh = nc.dram_tensor("whh", [128, KC, 4 * H], rdt, kind="ExternalInput")
    bias4 = nc.dram_tensor("bias4", [4, 2, 128], f16, kind="ExternalInput")
    ind4 = nc.dram_tensor("ind4", [4, 512], f16, kind="ExternalInput")
    wd = nc.dram_tensor("wd", [128, KC, 1], rdt, kind="ExternalInput")
    out = nc.dram_tensor("out", [1, 2, BLOC], f32, kind="ExternalOutput")

    with ExitStack() as ctx:
        tc = ctx.enter_context(TileContext(nc))
        consts = ctx.enter_context(tc.tile_pool(name="consts", bufs=1))
        state = ctx.enter_context(tc.tile_pool(name="state", bufs=1))
        xbp = ctx.enter_context(tc.tile_pool(name="xbp", bufs=1))
        ew = ctx.enter_context(tc.tile_pool(name="ew", bufs=3))
        gpsum = ctx.enter_context(tc.tile_pool(name="gpsum", bufs=2, space="PSUM"))
        hpsum = ctx.enter_context(tc.tile_pool(name="hpsum", bufs=1, space="PSUM"))

        # constants into SBUF
        wih_sb = consts.tile([128, 4 * H], f16)
        nc.sync.dma_start(out=wih_sb[:, :], in_=wih[:, :])
        whh_sb = consts.tile([128, KC, 4 * H], rdt)
        nc.sync.dma_start(out=whh_sb[:, :, :], in_=whh[:, :, :])
        bias_sb = consts.tile([4, 2, 128], f16)
        nc.sync.dma_start(out=bias_sb[:, :, :], in_=bias4[:, :, :])
        ind_sb = consts.tile([4, 512], f16)
        nc.sync.dma_start(out=ind_sb[:, :], in_=ind4[:, :])
        wd_sb = consts.tile([128, KC, 1], rdt)
        nc.sync.dma_start(out=wd_sb[:, :, :], in_=wd[:, :, :])

        hT = state.tile([128, KC, BLOC], rdt)
        # gcat packs [ghat(2 chunks), c(2 chunks)] so one tensor_mul computes
        # both i*ghat and f*c against the contiguous sigmoid outputs [i, f]
        gcat = state.tile([128, 4, BLOC], f32)
        nc.vector.memset(hT[:, :, :], 0.0)
        nc.vector.memset(gcat[:, :, :], 0.0)

        # one giant xbar-transpose of the whole input: [BLOC*T, D] -> [D, BLOC*T]
        # (rows of x are contiguous across the full (b, t) range)
        xTb = xbp.tile([128, BLOC, seq_len], f16)
        nc.sync.dma_start_transpose(
            out=xTb[:, :, :],
            in_=x[:, :, :].rearrange("b t d -> (b t) d"),
        )

        if True:
            for gi in range(seq_len // G4):
                t0 = gi * G4
                P = gpsum.tile([128, GC, G4, BLOC], f32, tag="gates")
                # bias fill: one K=4 matmul per PSUM bank (4 gate chunks each)
                nc.tensor.matmul(
                    P[:, 0:4, :, :], lhsT=bias_sb[:, 0, :], rhs=ind_sb[:, :],
                    start=True, stop=False, skip_group_check=True,
                )
                nc.tensor.matmul(
                    P[:, 4:8, :, :], lhsT=bias_sb[:, 1, :], rhs=ind_sb[:, :],
                    start=True, stop=False, skip_group_check=True,
                )
                # input projection for these 4 steps (strided t-major view of xTb)
                xvw = xTb[:, :, t0:t0 + G4].rearrange("p b t -> p t b")
                for gc in range(GC):
                    nc.tensor.matmul(
                        P[:, gc, :, :],
                        lhsT=wih_sb[:, gc * 128:(gc + 1) * 128],
                        rhs=xvw,
                        start=False, stop=False, skip_group_check=True,
                    )
                for tt in range(G4):
                    # recurrence matmuls: gates += W_hh^T @ h
                    for gc in range(GC):
                        for kc in range(KC):
                            nc.tensor.matmul(
                                P[:, gc, tt, :],
                                lhsT=whh_sb[:, kc, gc * 128:(gc + 1) * 128],
                                rhs=hT[:, kc, :],
                                start=False,
                                stop=(kc == KC - 1),
                                skip_group_check=True,
                            )
                    # elementwise cell update:
                    #   ghat = tanh(g); [i,f,o] = sigmoid(...)
                    #   prod = [i, f] * [ghat, c];  c = prod0 + prod1
                    #   h = o * tanh(c)
                    nc.scalar.activation(gcat[:, 0:2, :], P[:, 0:2, tt, :], AF.Tanh)
                    sb_ifo = ew.tile([128, 6, BLOC], f32, tag="sb_ifo")
                    nc.scalar.activation(sb_ifo[:, :, :], P[:, 2:8, tt, :], AF.Sigmoid)
                    prod = ew.tile([128, 4, BLOC], f32, tag="prod")
                    nc.vector.tensor_mul(prod[:, :, :], sb_ifo[:, 0:4, :], gcat[:, :, :])
                    nc.vector.tensor_add(gcat[:, 2:4, :], prod[:, 0:2, :], prod[:, 2:4, :])
                    thc = ew.tile([128, 2, BLOC], f32, tag="thc")
                    nc.scalar.activation(thc[:, :, :], gcat[:, 2:4, :], AF.Tanh)
                    nc.vector.tensor_mul(hT[:, :, :], sb_ifo[:, 4:6, :], thc[:, :, :])

        # head: d = h @ w_d + b_d ; probs = [sigmoid(d+bd), sigmoid(-d-bd)]
        hps = hpsum.tile([1, BLOC], f32)
        nc.tensor.matmul(hps[:, :], lhsT=wd_sb[:, 0, :], rhs=hT[:, 0, :],
                         start=True, stop=False, skip_group_check=True)
        nc.tensor.matmul(hps[:, :], lhsT=wd_sb[:, 1, :], rhs=hT[:, 1, :],
                         start=False, stop=True, skip_group_check=True)
        outsb = consts.tile([1, 2, BLOC], f32)
        bd_pos = consts.tile([1, 1], f32)
        bd_neg = consts.tile([1, 1], f32)
        nc.vector.memset(bd_pos[:, :], float(_cache["b_d"]))
        nc.vector.memset(bd_neg[:, :], -float(_cache["b_d"]))
        nc.scalar.activation(outsb[:, 0, :], hps[:, :], AF.Sigmoid,
                             bias=bd_pos[:, :], scale=1.0)
        nc.scalar.activation(outsb[:, 1, :], hps[:, :], AF.Sigmoid,
                             bias=bd_neg[:, :], scale=-1.0)
        nc.sync.dma_start(out=out[:, :, :], in_=outsb[:, :, :])

    nc.compile()
    return nc


def _prep_host(inputs, W_ih, W_hh, b_ih, b_hh, W_lin, b_lin):
    """Host-side weight preprocessing: gate permutation + transposed layouts."""
    # PyTorch gate row order [i, f, g, o] (256 each) -> chunk order
    # [g0, g1, i0, i1, f0, f1, o0, o1] (128-row chunks)
    perm = np.concatenate([
        np.arange(512, 768),    # g
        np.arange(0, 256),      # i
        np.arange(256, 512),    # f
        np.arange(768, 1024),   # o
    ])
    rnp = np.float16 if RECUR_F16 else np.float32

    Wih_p = np.ascontiguousarray(W_ih[perm])            # [1024, 128]
    Whh_p = np.ascontiguousarray(W_hh[perm])            # [1024, 256]
    b_p = (b_ih + b_hh)[perm].astype(np.float32)        # [1024]

    wih_host = np.ascontiguousarray(Wih_p.T).astype(np.float16)     # [128, 1024]
    whh_host = np.ascontiguousarray(
        Whh_p.T.reshape(KC, 128, 4 * H).transpose(1, 0, 2)
    ).astype(rnp)                                                   # [128, 2, 1024]
    # bias lhsT: bias4[k, bank, p] = b_p[(bank*4 + k)*128 + p]
    bias4 = np.ascontiguousarray(
        b_p.reshape(2, 4, 128).transpose(1, 0, 2)
    ).astype(np.float16)                                            # [4, 2, 128]
    ind4 = np.kron(np.eye(4, dtype=np.float16), np.ones((1, 128), np.float16))
    ind4 = np.ascontiguousarray(ind4)                               # [4, 512]
    w_d = (W_lin[0] - W_lin[1]).astype(np.float32)                  # [256]
    wd_host = np.ascontiguousarray(w_d.reshape(KC, 128).T.reshape(128, KC, 1)).astype(rnp)
    b_d = float(b_lin[0] - b_lin[1])

    x_f16 = inputs.astype(np.float16)                               # [256, T, 128]
    return x_f16, wih_host, whh_host, bias4, ind4, wd_host, b_d


def kernel(inputs, W_ih, W_hh, b_ih, b_hh, W_lin, b_lin):
    from concourse.bass_utils import run_bass_kernel_spmd

    inputs = np.asarray(inputs, dtype=np.float32)
    x_f16, wih_h, whh_h, bias4, ind4, wd_h, b_d = _prep_host(
        np.asarray(inputs), np.asarray(W_ih), np.asarray(W_hh),
        np.asarray(b_ih), np.asarray(b_hh), np.asarray(W_lin), np.asarray(b_lin))
    if _cache.get("b_d") != b_d or "nc" not in _cache:
        _cache["b_d"] = b_d
        _cache["nc"] = _build_program(T)
    nc = _cache["nc"]

    in_maps = []
    for j in range(NCORES):
        in_maps.append({
            "x": np.ascontiguousarray(x_f16[j * BLOC:(j + 1) * BLOC]),
            "wih": wih_h, "whh": whh_h, "bias4": bias4,
            "ind4": ind4, "wd": wd_h,
        })

    res = run_bass_kernel_spmd(nc, in_maps, core_ids=list(range(NCORES)))
    _cache["last_result"] = res
    out = np.concatenate(
        [np.asarray(r["out"])[0].T for r in res.results], axis=0)
    return np.ascontiguousarray(out).astype(np.float32)


# revision 18
# speedup vs baseline: 15.0566x; 15.0566x over previous
"""Trainium2 Bass kernel for BasicLSTM (nn_BasicLSTM_16320875724833).

Problem: inputs [256, 1024, 128] f32; LSTM(H=256) over T=1024 steps, then
linear [256->2] + softmax on the final hidden state. Output [256, 2] f32.

Strategy (8 cores, data-parallel over batch, 32 rows/core):
  - All state kept "transposed" (feature-major): hT/cT are [128p, 2, 32].
  - Gates for 4 consecutive steps accumulate in one PSUM tile
    [128p(gate-within-chunk), 8(gate-chunk), 4(step), 32(batch)]:
      1. bias matmul (K=4 indicator trick) fills bank with b_ih+b_hh, start=True
      2. input-projection matmuls (W_ih^T @ xT) accumulate, per gate-chunk
      3. per step, 16 recurrence matmuls (W_hh^T chunks @ hT) accumulate
  - x is cast to fp16 on host, DMA'd per-batch-row and transposed to
    d-major via the DMA xbar transpose engine.
  - Gate chunk order (host-side row permutation of the PyTorch [i,f,g,o]
    layout): [g0,g1, i0,i1, f0,f1, o0,o1] so tanh(g) is chunks 0:2 and
    sigmoid(i,f,o) is one contiguous ACT op over chunks 2:8.
  - Head: softmax over 2 classes == [sigmoid(d), sigmoid(-d)] with
    d = h @ (W_lin[0]-W_lin[1]) + (b_lin[0]-b_lin[1]).
"""

import numpy as np

# ---- problem constants (hardcoded; kernel.py must be self-contained) ----
B, T, D, H = 256, 1024, 128, 256
NCORES = 8
BLOC = B // NCORES          # 32 batch rows per core
GC = 8                      # gate chunks of 128 (4H = 1024)
KC = 2                      # hidden chunks of 128 (H = 256)
G4 = 4                      # timesteps per PSUM group
TC = 128                    # timesteps per transposed-x chunk

RECUR_F16 = True            # fp16 recurrence weights/state (False -> fp32)
W8 = True                   # fp8(e4m3) W_hh stationary operand (halves LDW time)
TAILSPLIT = False           # split tanh(c)/h-mul into kc halves + kc-major MMs
GBUFS = 2                   # gates PSUM pool buffers
REPEAT = 1                  # timing-only: run the recurrence REPEAT times

_cache = {}


def _build_program(seq_len=T):
    import concourse.bass as bass
    import concourse.mybir as mybir
    from concourse import bacc
    from concourse.tile import TileContext
    from contextlib import ExitStack

    f16 = mybir.dt.float16
    f32 = mybir.dt.float32
    rdt = f16 if RECUR_F16 else f32
    AF = mybir.ActivationFunctionType

    nc = bacc.Bacc(None, target_bir_lowering=False)

    x = nc.dram_tensor("x", [BLOC, seq_len, D], f16, kind="ExternalInput")
    wih = nc.dram_tensor("wih", [D, 4 * H], f16, kind="ExternalInput")
    wdt = mybir.dt.float8e4 if W8 else rdt
    whh = nc.dram_tensor("whh", [128, KC, 4 * H], wdt, kind="ExternalInput")
    bias4 = nc.dram_tensor("bias4", [4, 2, 128], f16, kind="ExternalInput")
    ind4 = nc.dram_tensor("ind4", [4, 512], f16, kind="ExternalInput")
    wd = nc.dram_tensor("wd", [128, KC, 1], rdt, kind="ExternalInput")
    out = nc.dram_tensor("out", [1, 2, BLOC], f32, kind="ExternalOutput")

    with ExitStack() as ctx:
        tc = ctx.enter_context(TileContext(nc))
        consts = ctx.enter_context(tc.tile_pool(name="consts", bufs=1))
        state = ctx.enter_context(tc.tile_pool(name="state", bufs=1))
        xbp = ctx.enter_context(tc.tile_pool(name="xbp", bufs=1))
        ew = ctx.enter_context(tc.tile_pool(name="ew", bufs=3))
        gpsum = ctx.enter_context(tc.tile_pool(name="gpsum", bufs=GBUFS, space="PSUM"))
        hpsum = ctx.enter_context(tc.tile_pool(name="hpsum", bufs=1, space="PSUM"))

        # constants into SBUF
        wih_sb = consts.tile([128, 4 * H], f16)
        nc.sync.dma_start(out=wih_sb[:, :], in_=wih[:, :])
        whh_sb = consts.tile([128, KC, 4 * H], wdt)
        nc.sync.dma_start(out=whh_sb[:, :, :], in_=whh[:, :, :])
        bias_sb = consts.tile([4, 2, 128], f16)
        nc.sync.dma_start(out=bias_sb[:, :, :], in_=bias4[:, :, :])
        ind_sb = consts.tile([4, 512], f16)
        nc.sync.dma_start(out=ind_sb[:, :], in_=ind4[:, :])
        wd_sb = consts.tile([128, KC, 1], rdt)
        nc.sync.dma_start(out=wd_sb[:, :, :], in_=wd[:, :, :])

        hT = state.tile([128, KC, BLOC], rdt)
        # gcat packs [ghat(2 chunks), c(2 chunks)] so one tensor_mul computes
        # both i*ghat and f*c against the contiguous sigmoid outputs [i, f]
        gcat = state.tile([128, 4, BLOC], f32)
        nc.vector.memset(hT[:, :, :], 0.0)
        nc.vector.memset(gcat[:, :, :], 0.0)

        # one giant xbar-transpose of the whole input: [BLOC*T, D] -> [D, BLOC*T]
        # (rows of x are contiguous across the full (b, t) range)
        xTb = xbp.tile([128, BLOC, seq_len], f16)
        nc.sync.dma_start_transpose(
            out=xTb[:, :, :],
            in_=x[:, :, :].rearrange("b t d -> (b t) d"),
        )

        for gi in range(REPEAT * seq_len // G4):
            if True:
                t0 = (gi * G4) % seq_len
                P = gpsum.tile([128, GC, G4, BLOC], f32, tag="gates")
                # bias fill: one K=4 matmul per PSUM bank (4 gate chunks each)
                nc.tensor.matmul(
                    P[:, 0:4, :, :], lhsT=bias_sb[:, 0, :], rhs=ind_sb[:, :],
                    start=True, stop=False, skip_group_check=True,
                )
                nc.tensor.matmul(
                    P[:, 4:8, :, :], lhsT=bias_sb[:, 1, :], rhs=ind_sb[:, :],
                    start=True, stop=False, skip_group_check=True,
                )
                # input projection for these 4 steps (strided t-major view of xTb)
                xvw = xTb[:, :, t0:t0 + G4].rearrange("p b t -> p t b")
                for gc in range(GC):
                    nc.tensor.matmul(
                        P[:, gc, :, :],
                        lhsT=wih_sb[:, gc * 128:(gc + 1) * 128],
                        rhs=xvw,
                        start=False, stop=False, skip_group_check=True,
                    )
                for tt in range(G4):
                    # recurrence matmuls: gates += W_hh^T @ h
                    if TAILSPLIT:
                        mm_order = [(gc, kc) for kc in range(KC) for gc in range(GC)]
                    else:
                        mm_order = [(gc, kc) for gc in range(GC) for kc in range(KC)]
                    for gc, kc in mm_order:
                        nc.tensor.matmul(
                            P[:, gc, tt, :],
                            lhsT=whh_sb[:, kc, gc * 128:(gc + 1) * 128],
                            rhs=hT[:, kc, :],
                            start=False,
                            stop=(kc == KC - 1),
                            skip_group_check=True,
                        )
                    # elementwise cell update:
                    #   ghat = tanh(g); [i,f,o] = sigmoid(...)
                    #   prod = [i, f] * [ghat, c];  c = prod0 + prod1
                    #   h = o * tanh(c)
                    nc.scalar.activation(gcat[:, 0:2, :], P[:, 0:2, tt, :], AF.Tanh)
                    sb_ifo = ew.tile([128, 6, BLOC], f32, tag="sb_ifo")
                    nc.scalar.activation(sb_ifo[:, :, :], P[:, 2:8, tt, :], AF.Sigmoid)
                    prod = ew.tile([128, 4, BLOC], f32, tag="prod")
                    nc.vector.tensor_mul(prod[:, :, :], sb_ifo[:, 0:4, :], gcat[:, :, :])
                    nc.vector.tensor_add(gcat[:, 2:4, :], prod[:, 0:2, :], prod[:, 2:4, :])
                    thc = ew.tile([128, 2, BLOC], f32, tag="thc")
                    if TAILSPLIT:
                        for hc in range(KC):
                            nc.scalar.activation(thc[:, hc:hc + 1, :],
                                                 gcat[:, 2 + hc:3 + hc, :], AF.Tanh)
                            nc.vector.tensor_mul(hT[:, hc, :],
                                                 sb_ifo[:, 4 + hc:5 + hc, :],
                                                 thc[:, hc:hc + 1, :])
                    else:
                        nc.scalar.activation(thc[:, :, :], gcat[:, 2:4, :], AF.Tanh)
                        nc.vector.tensor_mul(hT[:, :, :], sb_ifo[:, 4:6, :], thc[:, :, :])

        # head: d = h @ w_d + b_d ; probs = [sigmoid(d+bd), sigmoid(-d-bd)]
        hps = hpsum.tile([1, BLOC], f32)
        nc.tensor.matmul(hps[:, :], lhsT=wd_sb[:, 0, :], rhs=hT[:, 0, :],
                         start=True, stop=False, skip_group_check=True)
        nc.tensor.matmul(hps[:, :], lhsT=wd_sb[:, 1, :], rhs=hT[:, 1, :],
                         start=False, stop=True, skip_group_check=True)
        outsb = consts.tile([1, 2, BLOC], f32)
        bd_pos = consts.tile([1, 1], f32)
        bd_neg = consts.tile([1, 1], f32)
        nc.vector.memset(bd_pos[:, :], float(_cache["b_d"]))
        nc.vector.memset(bd_neg[:, :], -float(_cache["b_d"]))
        nc.scalar.activation(outsb[:, 0, :], hps[:, :], AF.Sigmoid,
                             bias=bd_pos[:, :], scale=1.0)
        nc.scalar.activation(outsb[:, 1, :], hps[:, :], AF.Sigmoid,
                             bias=bd_neg[:, :], scale=-1.0)
        nc.sync.dma_start(out=out[:, :, :], in_=outsb[:, :, :])

    nc.compile()
    return nc


def _prep_host(inputs, W_ih, W_hh, b_ih, b_hh, W_lin, b_lin):
    """Host-side weight preprocessing: gate permutation + transposed layouts."""
    # PyTorch gate row order [i, f, g, o] (256 each) -> chunk order
    # [g0, g1, i0, i1, f0, f1, o0, o1] (128-row chunks)
    perm = np.concatenate([
        np.arange(512, 768),    # g
        np.arange(0, 256),      # i
        np.arange(256, 512),    # f
        np.arange(768, 1024),   # o
    ])
    rnp = np.float16 if RECUR_F16 else np.float32
    import concourse.mybir as _mb
    wnp = _mb.dt.np(_mb.dt.float8e4) if W8 else rnp

    Wih_p = np.ascontiguousarray(W_ih[perm])            # [1024, 128]
    Whh_p = np.ascontiguousarray(W_hh[perm])            # [1024, 256]
    b_p = (b_ih + b_hh)[perm].astype(np.float32)        # [1024]

    wih_host = np.ascontiguousarray(Wih_p.T).astype(np.float16)     # [128, 1024]
    whh_host = np.ascontiguousarray(
        Whh_p.T.reshape(KC, 128, 4 * H).transpose(1, 0, 2)
    ).astype(wnp)                                                   # [128, 2, 1024]
    # bias lhsT: bias4[k, bank, p] = b_p[(bank*4 + k)*128 + p]
    bias4 = np.ascontiguousarray(
        b_p.reshape(2, 4, 128).transpose(1, 0, 2)
    ).astype(np.float16)                                            # [4, 2, 128]
    ind4 = np.kron(np.eye(4, dtype=np.float16), np.ones((1, 128), np.float16))
    ind4 = np.ascontiguousarray(ind4)                               # [4, 512]
    w_d = (W_lin[0] - W_lin[1]).astype(np.float32)                  # [256]
    wd_host = np.ascontiguousarray(w_d.reshape(KC, 128).T.reshape(128, KC, 1)).astype(rnp)
    b_d = float(b_lin[0] - b_lin[1])

    x_f16 = inputs.astype(np.float16)                               # [256, T, 128]
    return x_f16, wih_host, whh_host, bias4, ind4, wd_host, b_d


def kernel(inputs, W_ih, W_hh, b_ih, b_hh, W_lin, b_lin):
    from concourse.bass_utils import run_bass_kernel_spmd

    inputs = np.asarray(inputs, dtype=np.float32)
    x_f16, wih_h, whh_h, bias4, ind4, wd_h, b_d = _prep_host(
        np.asarray(inputs), np.asarray(W_ih), np.asarray(W_hh),
        np.asarray(b_ih), np.asarray(b_hh), np.asarray(W_lin), np.asarray(b_lin))
    if _cache.get("b_d") != b_d or "nc" not in _cache:
        _cache["b_d"] = b_d
        _cache["nc"] = _build_program(T)
    nc = _cache["nc"]

    in_maps = []
    for j in range(NCORES):
        in_maps.append({
            "x": np.ascontiguousarray(x_f16[j * BLOC:(j + 1) * BLOC]),
            "wih": wih_h, "whh": whh_h, "bias4": bias4,
            "ind4": ind4, "wd": wd_h,
        })

    res = run_bass_kernel_spmd(nc, in_maps, core_ids=list(range(NCORES)))
    _cache["last_result"] = res
    out = np.concatenate(
        [np.asarray(r["out"])[0].T for r in res.results], axis=0)
    return np.ascontiguousarray(out).astype(np.float32)
